# revision 15
# baseline (speedup 1.0000x reference)
"""Trainium2 Bass kernel for the spike-decoder GNN message-passing module.

Math (per batch b, output time tau in [0, T-2], variable v):
  out[b,tau,v] = bias[v]
               + sum_{i,k} w[v,i,k] * x[b,i,tau+k-(K-2)]          (static conv)
               + sum_{e: recv[e]=v} sum_k dw[e,b,tau,k] * x[b,send[e],tau+k-(K-2)]
with w = conv_weight masked at w[i,i,K-1] = 0, x = spikes[...,0] transposed to
[b, nvar, t], and out-of-range x treated as zero.

Sharding: 8 cores = (b in 0..3) x (time half h in 0..1). Each core computes a
1024-wide tau window ([0,1024) or [1023,2047) — one overlapping column keeps
shapes uniform for SPMD).

dyn_weights streams as bf16 (exact relative to an fp32 kernel: x is 0/1, so
the masked products round to bf16(dw) either way and the PE consumed bf16
products already). ~17.8 MB/core at ~380 GB/s makes the DMA stream (~46 us)
the wall; everything else hides under it. fp8 for the conv weights was tried
and rejected: e4m3's 3-bit mantissa puts the static conv's max error at
~3e-2 of output scale (tolerance 2e-2). fp8 products would halve the scatter
matmul time but drop the DVE to its 1x mode (the 2x path needs 16-bit
operands including outputs), so bf16 is the floor.

On-core algorithm:
  - xg[e,:] = x[send[e],:] gathered via one-hot matmul on PE (exact: x is 0/1),
    kept in two bf16 copies (xgA, xgB = xgA shifted left 1) so every sliding
    window the DVE reads starts 4B-aligned — that keeps tensor_tensor in its
    2x bf16 perf mode (odd-k windows would otherwise be 2-byte aligned -> 1x).
  - products P[e,(k,tau)] = dwt * window(xg) on DVE: per k-group one bf16
    tensor_mul (even k's from xgA, odd k's from xgB), 3D APs stride-1 in tau.
  - k-reduction + recv-scatter + transpose folded into PE: for each k, a bf16
    matmul with stationary one-hot recv matrix and moving operand = P's
    k-slice, accumulating into PSUM[v, tau]
  - static conv: 16 bf16 matmuls with stationary wT_k and shifted xpad slices
    (the shifted copy keeps odd-k moving operands 4B-aligned; odd-offset bf16
    moving operands measurably run ~60% slower on the PE). The shifted copy
    is built on ScalarE during the DMA shadow rather than shipped.
  - bias: rank-1 matmul (bias x ones)
All terms accumulate into one PSUM bank [v, 512], copied out by ScalarE as
bf16 and DMA'd back. Host transposes/upcasts while assembling the result.

Pipelining: everything rides ONE sync-queue DMA stream, constants ordered
between the first dw tile and the rest (a second engine queue starves: HW DMA
engines arbitrate per-descriptor and a fat stream crushes a thin queue).
Full-tile 16KB-row DMAs (8KB rows measured ~10% slower). Tile 0 is split into
half-DMAs + half-groups so the first multiply starts early; tile 7 is split
[12k|4k] so the tail after the last DMA byte is short. The DVE is
arrival-gated throughout (~4.6us service vs ~5.2us arrival per tile), so the
wall is preamble (~6.7us, fixed) + stream (~46us) + last-piece drain
(~8us) + postamble (~2.5us).
"""

import numpy as np

B, T, NVAR, K, E = 4, 2048, 128, 16, 512
TAU = T - 1            # 2047
L = 1024               # per-core tau window
NC_COUNT = 8
W_XPAD = L + K         # 1040 (1039 used; padded even)
ETILES = E // 128      # 4
CHUNK = 512            # tau chunk per PSUM bank
NCHUNK = L // CHUNK    # 2

_PROGRAM = None


def _build_program():
    import concourse.bass as bass
    import concourse.bacc as bacc
    import concourse.mybir as mybir
    import concourse.tile as tile

    f32 = mybir.dt.float32
    f32r = mybir.dt.float32r
    bf16 = mybir.dt.bfloat16
    # Bacc (not plain Bass): its compile pipeline runs generate_event_semaphores,
    # which splits multi-semaphore waits — a raw fp32 Matmult supports only one
    # sync-wait slot and walrus rejects more ("Too many sync wait commands").
    nc = bacc.Bacc()

    xpad_d = nc.declare_dram_parameter("xpad", [NVAR, W_XPAD], bf16, isOutput=False)
    dw_d = nc.declare_dram_parameter("dw", [NCHUNK * E, CHUNK * K], bf16, isOutput=False)
    ssend_d = nc.declare_dram_parameter("ssend", [NVAR, E], bf16, isOutput=False)
    wt_d = nc.declare_dram_parameter("wt", [NVAR, K * NVAR], bf16, isOutput=False)
    recv_d = nc.declare_dram_parameter("recvT", [128, ETILES * NVAR], bf16, isOutput=False)
    bo_d = nc.declare_dram_parameter("bias_ones", [1, NVAR + CHUNK], f32r, isOutput=False)
    y_d = nc.declare_dram_parameter("yT", [NVAR, L], bf16, isOutput=True)

    with tile.TileContext(nc) as tc:
        with (
            tc.tile_pool(name="consts", bufs=1) as consts,
            tc.tile_pool(name="xgp", bufs=1) as xgp,
            tc.tile_pool(name="gpsum", bufs=2, space=bass.MemorySpace.PSUM) as gpsum,
            tc.tile_pool(name="dwp", bufs=4) as dwp,
            tc.tile_pool(name="prodp", bufs=3) as prodp,
            tc.tile_pool(name="opsum", bufs=2, space=bass.MemorySpace.PSUM) as opsum,
            tc.tile_pool(name="resp", bufs=2) as resp,
        ):
            NT = NCHUNK * ETILES  # 8 dw tiles
            HK = CHUNK * K // 2   # half-tile product columns (4096)

            # SP/HWDGE issue order = completion order (per-engine FIFO):
            # gather inputs first (small), then the dw stream owns the queue.
            xpad = consts.tile([NVAR, 2 * W_XPAD], bf16)
            nc.sync.dma_start(xpad[:, 0:W_XPAD], xpad_d[:])
            # shifted-left-by-one copy for 4B-aligned odd-k static windows,
            # built on ScalarE during the DMA shadow (saves 0.27 MB of HBM)
            nc.scalar.copy(xpad[:, W_XPAD:2 * W_XPAD - 1], xpad[:, 1:W_XPAD])
            ssend = consts.tile([NVAR, E], bf16)
            nc.sync.dma_start(ssend[:], ssend_d[:])

            def dw_dma(dwt, ti, bounds):
                h2, et = divmod(ti, ETILES)
                r0 = h2 * E + et * 128
                for a, b in zip(bounds[:-1], bounds[1:]):
                    nc.sync.dma_start(
                        dwt[:, a * CHUNK:b * CHUNK],
                        dw_d[r0:r0 + 128, a * CHUNK:b * CHUNK],
                    )

            dwt_tiles = []
            for ti in range(NT):
                dwt = dwp.tile([128, CHUNK * K], bf16, name="dwt", tag="dwt")
                dwt_tiles.append(dwt)
            dw_dma(dwt_tiles[0], 0, bounds=[0, 8, 16])
            # remaining small constants slot in behind the first dw tile
            wt = consts.tile([NVAR, K * NVAR], bf16)
            nc.sync.dma_start(wt[:], wt_d[:])
            recvT = consts.tile([128, ETILES * NVAR], bf16)
            nc.sync.dma_start(recvT[:], recv_d[:])
            bias_ones = consts.tile([1, NVAR + CHUNK], f32r)
            nc.sync.dma_start(bias_ones[:], bo_d[:])
            for ti in range(1, NT):
                dw_dma(dwt_tiles[ti], ti, bounds=([0, 12, 16] if ti == NT - 1 else [0, 16]))

            # Gather sender rows: xgA[et][p, j] = xpad[send[et*128+p], j],
            # xgB = xgA shifted left by one column (for 4B-aligned odd-k
            # windows). A-copies from PSUM on ScalarE; B-copy SBUF->SBUF.
            xgA, xgB = [], []
            for et in range(ETILES):
                xga = xgp.tile([128, W_XPAD], bf16, name=f"xga{et}", tag=f"xga{et}")
                xgb = xgp.tile([128, W_XPAD], bf16, name=f"xgb{et}", tag=f"xgb{et}")
                for j0 in range(0, W_XPAD, CHUNK):
                    jw = min(CHUNK, W_XPAD - j0)
                    gps = gpsum.tile([128, CHUNK], f32, name="gps", tag="gps")
                    nc.tensor.matmul(
                        gps[:, :jw],
                        ssend[:, et * 128:(et + 1) * 128],
                        xpad[:, j0:j0 + jw],
                        start=True, stop=True,
                    )
                    nc.scalar.copy(xga[:, j0:j0 + jw], gps[:, :jw])
                nc.scalar.copy(xgb[:, 0:W_XPAD - 1], xga[:, 1:W_XPAD])
                xgA.append(xga)
                xgB.append(xgb)

            ops_tiles = []
            for h2 in range(NCHUNK):
                o = opsum.tile([128, CHUNK], f32, name=f"ops{h2}", tag=f"ops{h2}")
                ops_tiles.append(o)

            def static_mm(h2, k, start=False):
                t0 = h2 * CHUNK
                # odd k reads the shifted copy so the bf16 moving operand
                # stays 4B-aligned (odd offsets measurably slow the PE)
                off = W_XPAD + t0 + k - 1 if (k % 2) else t0 + k
                nc.tensor.matmul(
                    ops_tiles[h2][:],
                    wt[:, k * NVAR:(k + 1) * NVAR],
                    xpad[:, off:off + CHUNK],
                    start=start, stop=False,
                )

            def bias_mm(h2):
                nc.tensor.matmul(
                    ops_tiles[h2][:],
                    bias_ones[:1, 0:NVAR],
                    bias_ones[:1, NVAR:NVAR + CHUNK],
                    start=False, stop=False,
                )

            # chunk-0 static conv + bias up front (PE warmup while dw streams)
            for k in range(K):
                static_mm(0, k, start=(k == 0))
            bias_mm(0)

            # chunk-1 static matmuls fill PE gaps across the early groups
            fill = [("s", k) for k in range(K)] + [("b", None)]
            fills_per_group = [2, 2, 2, 2, 2, 2, 2, 3, 0, 0]

            def do_ks(h2, et, dwt, pt, ks):
                """One DVE tensor_mul over the k-set `ks` (uniform step 2),
                then the PE scatter matmuls for those k's."""
                t0 = h2 * CHUNK
                drow = dwt.tensor.shape[-1]
                prow = pt.tensor.shape[-1]
                par = ks[0] % 2  # 0 -> xgA, 1 -> xgB
                xg = xgB[et] if par else xgA[et]
                xrow = xg.tensor.shape[-1]
                nk = len(ks)
                in0 = bass.AP(dwt.tensor, ks[0] * CHUNK,
                              [[drow, 128], [2 * CHUNK, nk], [1, CHUNK]])
                # window: in1[p, j, tau] = xg[p, t0 + ks[0]+2j - par + tau]
                in1 = bass.AP(xg.tensor, t0 + ks[0] - par,
                              [[xrow, 128], [2, nk], [1, CHUNK]])
                out3 = bass.AP(pt.tensor, ks[0] * CHUNK,
                               [[prow, 128], [2 * CHUNK, nk], [1, CHUNK]])
                nc.vector.tensor_mul(out3, in0, in1)
                for k in ks:
                    rhs = bass.AP(pt.tensor, k * CHUNK,
                                  [[prow, 128], [1, CHUNK]])
                    nc.tensor.matmul(
                        ops_tiles[h2][:],
                        recvT[:, et * NVAR:(et + 1) * NVAR],
                        rhs,
                        start=False,
                        stop=(et == ETILES - 1 and k == K - 1),
                    )

            # groups: tile 0 split in k-halves (starts on the first half-DMA),
            # tiles 1..6 whole, tile 7 split [12|4] (short tail after the
            # last DMA byte). Each group = [even k's, odd k's].
            def kgroups(ti):
                if ti == 0:
                    return [range(0, 8), range(8, 16)]
                if ti == NT - 1:
                    return [range(0, 12), range(12, 16)]
                return [range(0, 16)]

            groups = []
            for ti in range(NT):
                for rng in kgroups(ti):
                    groups.append((ti, [list(rng)[0::2], list(rng)[1::2]]))

            pts = {}
            for gi, (ti, ksets) in enumerate(groups):
                h2, et = divmod(ti, ETILES)
                dwt = dwt_tiles[ti]
                if ti not in pts:
                    pts[ti] = prodp.tile([128, CHUNK * K], bf16, name="pt", tag="pt")
                pt = pts[ti]
                last_of_tile = (gi == len(groups) - 1) or (groups[gi + 1][0] != ti)
                for ks in ksets:
                    do_ks(h2, et, dwt, pt, ks)
                for _ in range(fills_per_group[gi]):
                    kind, k = fill.pop(0)
                    if kind == "s":
                        static_mm(1, k, start=(k == 0))
                    else:
                        bias_mm(1)
                if et == ETILES - 1 and last_of_tile:
                    t0 = h2 * CHUNK
                    res = resp.tile([128, CHUNK], bf16, name="res", tag="res")
                    nc.scalar.copy(res[:], ops_tiles[h2][:])
                    nc.sync.dma_start(y_d[:, t0:t0 + CHUNK], res[:])

    nc.compile()
    return nc


def _get_program():
    global _PROGRAM
    if _PROGRAM is None:
        _PROGRAM = _build_program()
    return _PROGRAM


def _host_prep(spikes, conv_weight, conv_bias, dyn_weights, edge_send, edge_recv):
    import ml_dtypes
    bf16 = ml_dtypes.bfloat16

    spikes = np.asarray(spikes, dtype=np.float32)
    conv_weight = np.asarray(conv_weight, dtype=np.float32)
    conv_bias = np.asarray(conv_bias, dtype=np.float32)
    dyn_weights = np.asarray(dyn_weights)
    edge_send = np.asarray(edge_send, dtype=np.int64)
    edge_recv = np.asarray(edge_recv, dtype=np.int64)

    x = np.ascontiguousarray(spikes[..., 0].transpose(0, 2, 1))  # [B, NVAR, T]

    ssend = np.zeros((NVAR, E), bf16)
    ssend[edge_send, np.arange(E)] = 1.0

    recvT = np.zeros((128, ETILES * NVAR), bf16)
    for et in range(ETILES):
        rr = edge_recv[et * 128:(et + 1) * 128]
        recvT[np.arange(128), et * NVAR + rr] = 1.0

    w = conv_weight.copy()
    w[np.arange(NVAR), np.arange(NVAR), K - 1] = 0.0
    wt = np.ascontiguousarray(w.transpose(1, 2, 0)).reshape(NVAR, K * NVAR).astype(bf16)

    bias_ones = np.concatenate(
        [conv_bias, np.ones(CHUNK, np.float32)]
    ).reshape(1, NVAR + CHUNK).astype(np.float32)

    dwb = dyn_weights.astype(bf16)  # [E, B, T-1, K]

    in_maps = []
    for core in range(NC_COUNT):
        b, h = divmod(core, 2)
        tau0 = 0 if h == 0 else TAU - L  # 0 or 1023
        xpad2 = np.zeros((NVAR, W_XPAD), np.float32)
        lo = tau0 - (K - 2)  # first x column needed
        src_lo = max(lo, 0)
        xpad2[:, src_lo - lo:W_XPAD - 1] = x[b, :, src_lo:tau0 + L + 1]
        a = dwb[:, b, tau0:tau0 + L, :]                  # [E, L, K]
        a = a.reshape(E, NCHUNK, CHUNK, K)               # [E, h2, tau, k]
        a = a.transpose(1, 0, 3, 2)                      # [h2, E, k, tau]
        dw = np.ascontiguousarray(a).reshape(NCHUNK * E, CHUNK * K)
        in_maps.append({
            "xpad": xpad2.astype(bf16),
            "dw": dw,
            "ssend": ssend,
            "wt": wt,
            "recvT": recvT,
            "bias_ones": bias_ones,
        })
    return in_maps


def _assemble(results):
    out = np.empty((B, TAU, NVAR, 1), np.float32)
    for core in range(NC_COUNT):
        b, h = divmod(core, 2)
        yT = results[core]["yT"]  # [NVAR, L] bf16
        if h == 0:
            out[b, 0:L, :, 0] = yT.T
        else:
            out[b, L:TAU, :, 0] = yT[:, 1:L].T
    return out


def run_on_hw(in_maps, trace=False, **kwargs):
    from concourse.bass_utils import run_bass_kernel_spmd

    nc = _get_program()
    return run_bass_kernel_spmd(
        nc, in_maps, core_ids=list(range(NC_COUNT)), trace=trace, **kwargs
    )


def kernel(spikes, conv_weight, conv_bias, dyn_weights, edge_send, edge_recv):
    in_maps = _host_prep(
        spikes, conv_weight, conv_bias, dyn_weights, edge_send, edge_recv
    )
    res = run_on_hw(in_maps)
    return _assemble(res.results)


# revision 16
# speedup vs baseline: 1.1181x; 1.1181x over previous
"""Trainium2 Bass kernel for the spike-decoder GNN message-passing module.

Math (per batch b, output time tau in [0, T-2], variable v):
  out[b,tau,v] = bias[v]
               + sum_{i,k} w[v,i,k] * x[b,i,tau+k-(K-2)]          (static conv)
               + sum_{e: recv[e]=v} sum_k dw[e,b,tau,k] * x[b,send[e],tau+k-(K-2)]
with w = conv_weight masked at w[i,i,K-1] = 0, x = spikes[...,0] transposed to
[b, nvar, t], and out-of-range x treated as zero.

Sharding: 8 cores = (b in 0..3) x (time half h in 0..1). Each core computes a
1024-wide tau window ([0,1024) or [1023,2047) — one overlapping column keeps
shapes uniform for SPMD).

dyn_weights streams as bf16 (exact relative to an fp32 kernel: x is 0/1, so
the masked products round to bf16(dw) either way and the PE consumed bf16
products already). ~17.8 MB/core at ~380 GB/s makes the DMA stream (~46 us)
the wall; everything else hides under it. fp8 for the conv weights was tried
and rejected: e4m3's 3-bit mantissa puts the static conv's max error at
~3e-2 of output scale (tolerance 2e-2). fp8 products would halve the scatter
matmul time but drop the DVE to its 1x mode (the 2x path needs 16-bit
operands including outputs), so bf16 is the floor.

On-core algorithm:
  - xg[e,:] = x[send[e],:] gathered via one-hot matmul on PE (exact: x is 0/1),
    kept in two bf16 copies (xgA, xgB = xgA shifted left 1) so every sliding
    window the DVE reads starts 4B-aligned — that keeps tensor_tensor in its
    2x bf16 perf mode (odd-k windows would otherwise be 2-byte aligned -> 1x).
  - products P[e,(k,tau)] = dwt * window(xg) on DVE: per k-group one bf16
    tensor_mul (even k's from xgA, odd k's from xgB), 3D APs stride-1 in tau.
  - k-reduction + recv-scatter + transpose folded into PE: for each k, a bf16
    matmul with stationary one-hot recv matrix and moving operand = P's
    k-slice, accumulating into PSUM[v, tau]
  - static conv: 16 bf16 matmuls with stationary wT_k and shifted xpad slices
    (the shifted copy keeps odd-k moving operands 4B-aligned; odd-offset bf16
    moving operands measurably run ~60% slower on the PE). The shifted copy
    is built on ScalarE during the DMA shadow rather than shipped.
  - bias: rank-1 matmul (bias x ones)
All terms accumulate into one PSUM bank [v, 512], copied out by ScalarE as
bf16 and DMA'd back. Host transposes/upcasts while assembling the result.

Pipelining: everything rides ONE sync-queue DMA stream, constants ordered
between the first dw tile and the rest (a second engine queue starves: HW DMA
engines arbitrate per-descriptor and a fat stream crushes a thin queue).
Full-tile 16KB-row DMAs (8KB rows measured ~10% slower). Tile 0 is split into
half-DMAs + half-groups so the first multiply starts early; tile 7 is split
[12k|4k] so the tail after the last DMA byte is short. The DVE is
arrival-gated throughout (~4.6us service vs ~5.2us arrival per tile), so the
wall is preamble (~6.7us, fixed) + stream (~46us) + last-piece drain
(~8us) + postamble (~2.5us).
"""

import numpy as np

B, T, NVAR, K, E = 4, 2048, 128, 16, 512
TAU = T - 1            # 2047
L = 1024               # per-core tau window
NC_COUNT = 8
W_XPAD = L + K         # 1040 (1039 used; padded even)
ETILES = E // 128      # 4
CHUNK = 512            # tau chunk per PSUM bank
NCHUNK = L // CHUNK    # 2

_PROGRAM = None


def _build_program():
    import concourse.bass as bass
    import concourse.bacc as bacc
    import concourse.mybir as mybir
    import concourse.tile as tile

    f32 = mybir.dt.float32
    f32r = mybir.dt.float32r
    bf16 = mybir.dt.bfloat16
    # Bacc (not plain Bass): its compile pipeline runs generate_event_semaphores,
    # which splits multi-semaphore waits — a raw fp32 Matmult supports only one
    # sync-wait slot and walrus rejects more ("Too many sync wait commands").
    nc = bacc.Bacc()

    xpad_d = nc.declare_dram_parameter("xpad", [NVAR, W_XPAD], bf16, isOutput=False)
    dw_d = nc.declare_dram_parameter("dw", [NCHUNK * E, CHUNK * K], bf16, isOutput=False)
    ssend_d = nc.declare_dram_parameter("ssend", [NVAR, E], bf16, isOutput=False)
    wt_d = nc.declare_dram_parameter("wt", [NVAR, K * NVAR], bf16, isOutput=False)
    recv_d = nc.declare_dram_parameter("recvT", [128, ETILES * NVAR], bf16, isOutput=False)
    bo_d = nc.declare_dram_parameter("bias_ones", [1, NVAR + CHUNK], f32r, isOutput=False)
    y_d = nc.declare_dram_parameter("yT", [NVAR, L], bf16, isOutput=True)

    with tile.TileContext(nc) as tc:
        with (
            tc.tile_pool(name="consts", bufs=1) as consts,
            tc.tile_pool(name="xgp", bufs=1) as xgp,
            tc.tile_pool(name="gpsum", bufs=2, space=bass.MemorySpace.PSUM) as gpsum,
            tc.tile_pool(name="dwp", bufs=4) as dwp,
            tc.tile_pool(name="prodp", bufs=3) as prodp,
            tc.tile_pool(name="opsum", bufs=2, space=bass.MemorySpace.PSUM) as opsum,
            tc.tile_pool(name="resp", bufs=2) as resp,
        ):
            NT = NCHUNK * ETILES  # 8 dw tiles
            HK = CHUNK * K // 2   # half-tile product columns (4096)

            # SP/HWDGE issue order = completion order (per-engine FIFO):
            # gather inputs first (small), then the dw stream owns the queue.
            xpad = consts.tile([NVAR, 2 * W_XPAD], bf16)
            nc.sync.dma_start(xpad[:, 0:W_XPAD], xpad_d[:])
            # shifted-left-by-one copy for 4B-aligned odd-k static windows,
            # built on ScalarE during the DMA shadow (saves 0.27 MB of HBM)
            nc.scalar.copy(xpad[:, W_XPAD:2 * W_XPAD - 1], xpad[:, 1:W_XPAD])
            ssend = consts.tile([NVAR, E], bf16)
            nc.sync.dma_start(ssend[:], ssend_d[:])

            def dw_dma(dwt, ti, bounds):
                h2, et = divmod(ti, ETILES)
                r0 = h2 * E + et * 128
                for a, b in zip(bounds[:-1], bounds[1:]):
                    nc.sync.dma_start(
                        dwt[:, a * CHUNK:b * CHUNK],
                        dw_d[r0:r0 + 128, a * CHUNK:b * CHUNK],
                    )

            dwt_tiles = []
            for ti in range(NT):
                dwt = dwp.tile([128, CHUNK * K], bf16, name="dwt", tag="dwt")
                dwt_tiles.append(dwt)
            dw_dma(dwt_tiles[0], 0, bounds=[0, 8, 16])
            # remaining small constants slot in behind the first dw tile
            wt = consts.tile([NVAR, K * NVAR], bf16)
            nc.sync.dma_start(wt[:], wt_d[:])
            recvT = consts.tile([128, ETILES * NVAR], bf16)
            nc.sync.dma_start(recvT[:], recv_d[:])
            bias_ones = consts.tile([1, NVAR + CHUNK], f32r)
            nc.sync.dma_start(bias_ones[:], bo_d[:])
            for ti in range(1, NT):
                dw_dma(dwt_tiles[ti], ti, bounds=([0, 14, 16] if ti == NT - 1 else [0, 16]))

            # Gather sender rows: xgA[et][p, j] = xpad[send[et*128+p], j],
            # xgB = xgA shifted left by one column (for 4B-aligned odd-k
            # windows). A-copies from PSUM on ScalarE; B-copy SBUF->SBUF.
            xgA, xgB = [], []
            for et in range(ETILES):
                xga = xgp.tile([128, W_XPAD], bf16, name=f"xga{et}", tag=f"xga{et}")
                xgb = xgp.tile([128, W_XPAD], bf16, name=f"xgb{et}", tag=f"xgb{et}")
                for j0 in range(0, W_XPAD, CHUNK):
                    jw = min(CHUNK, W_XPAD - j0)
                    gps = gpsum.tile([128, CHUNK], f32, name="gps", tag="gps")
                    nc.tensor.matmul(
                        gps[:, :jw],
                        ssend[:, et * 128:(et + 1) * 128],
                        xpad[:, j0:j0 + jw],
                        start=True, stop=True,
                    )
                    nc.scalar.copy(xga[:, j0:j0 + jw], gps[:, :jw])
                nc.scalar.copy(xgb[:, 0:W_XPAD - 1], xga[:, 1:W_XPAD])
                xgA.append(xga)
                xgB.append(xgb)

            ops_tiles = []
            for h2 in range(NCHUNK):
                o = opsum.tile([128, CHUNK], f32, name=f"ops{h2}", tag=f"ops{h2}")
                ops_tiles.append(o)

            def static_mm(h2, k, start=False):
                t0 = h2 * CHUNK
                # odd k reads the shifted copy so the bf16 moving operand
                # stays 4B-aligned (odd offsets measurably slow the PE)
                off = W_XPAD + t0 + k - 1 if (k % 2) else t0 + k
                nc.tensor.matmul(
                    ops_tiles[h2][:],
                    wt[:, k * NVAR:(k + 1) * NVAR],
                    xpad[:, off:off + CHUNK],
                    start=start, stop=False,
                )

            def bias_mm(h2):
                nc.tensor.matmul(
                    ops_tiles[h2][:],
                    bias_ones[:1, 0:NVAR],
                    bias_ones[:1, NVAR:NVAR + CHUNK],
                    start=False, stop=False,
                )

            # chunk-0 static conv + bias up front (PE warmup while dw streams)
            for k in range(K):
                static_mm(0, k, start=(k == 0))
            bias_mm(0)

            # chunk-1 static matmuls fill PE gaps across the early groups
            fill = [("s", k) for k in range(K)] + [("b", None)]
            fills_per_group = [2, 2, 2, 2, 2, 2, 2, 3, 0, 0]

            def do_ks(h2, et, dwt, pt, ks):
                """One DVE tensor_mul over the k-set `ks` (uniform step 2),
                then the PE scatter matmuls for those k's."""
                t0 = h2 * CHUNK
                drow = dwt.tensor.shape[-1]
                prow = pt.tensor.shape[-1]
                par = ks[0] % 2  # 0 -> xgA, 1 -> xgB
                xg = xgB[et] if par else xgA[et]
                xrow = xg.tensor.shape[-1]
                nk = len(ks)
                in0 = bass.AP(dwt.tensor, ks[0] * CHUNK,
                              [[drow, 128], [2 * CHUNK, nk], [1, CHUNK]])
                # window: in1[p, j, tau] = xg[p, t0 + ks[0]+2j - par + tau]
                in1 = bass.AP(xg.tensor, t0 + ks[0] - par,
                              [[xrow, 128], [2, nk], [1, CHUNK]])
                out3 = bass.AP(pt.tensor, ks[0] * CHUNK,
                               [[prow, 128], [2 * CHUNK, nk], [1, CHUNK]])
                nc.vector.tensor_mul(out3, in0, in1)
                for k in ks:
                    rhs = bass.AP(pt.tensor, k * CHUNK,
                                  [[prow, 128], [1, CHUNK]])
                    nc.tensor.matmul(
                        ops_tiles[h2][:],
                        recvT[:, et * NVAR:(et + 1) * NVAR],
                        rhs,
                        start=False,
                        stop=(et == ETILES - 1 and k == K - 1),
                    )

            # groups: tile 0 split in k-halves (starts on the first half-DMA),
            # tiles 1..6 whole, tile 7 split [12|4] (short tail after the
            # last DMA byte). Each group = [even k's, odd k's].
            def kgroups(ti):
                if ti == 0:
                    return [range(0, 8), range(8, 16)]
                if ti == NT - 1:
                    return [range(0, 14), range(14, 16)]
                return [range(0, 16)]

            groups = []
            for ti in range(NT):
                for rng in kgroups(ti):
                    groups.append((ti, [list(rng)[0::2], list(rng)[1::2]]))

            pts = {}
            for gi, (ti, ksets) in enumerate(groups):
                h2, et = divmod(ti, ETILES)
                dwt = dwt_tiles[ti]
                if ti not in pts:
                    pts[ti] = prodp.tile([128, CHUNK * K], bf16, name="pt", tag="pt")
                pt = pts[ti]
                last_of_tile = (gi == len(groups) - 1) or (groups[gi + 1][0] != ti)
                for ks in ksets:
                    do_ks(h2, et, dwt, pt, ks)
                for _ in range(fills_per_group[gi]):
                    kind, k = fill.pop(0)
                    if kind == "s":
                        static_mm(1, k, start=(k == 0))
                    else:
                        bias_mm(1)
                if et == ETILES - 1 and last_of_tile:
                    t0 = h2 * CHUNK
                    res = resp.tile([128, CHUNK], bf16, name="res", tag="res")
                    nc.scalar.copy(res[:], ops_tiles[h2][:])
                    nc.sync.dma_start(y_d[:, t0:t0 + CHUNK], res[:])

    nc.compile()
    return nc


def _get_program():
    global _PROGRAM
    if _PROGRAM is None:
        _PROGRAM = _build_program()
    return _PROGRAM


def _host_prep(spikes, conv_weight, conv_bias, dyn_weights, edge_send, edge_recv):
    import ml_dtypes
    bf16 = ml_dtypes.bfloat16

    spikes = np.asarray(spikes, dtype=np.float32)
    conv_weight = np.asarray(conv_weight, dtype=np.float32)
    conv_bias = np.asarray(conv_bias, dtype=np.float32)
    dyn_weights = np.asarray(dyn_weights)
    edge_send = np.asarray(edge_send, dtype=np.int64)
    edge_recv = np.asarray(edge_recv, dtype=np.int64)

    x = np.ascontiguousarray(spikes[..., 0].transpose(0, 2, 1))  # [B, NVAR, T]

    ssend = np.zeros((NVAR, E), bf16)
    ssend[edge_send, np.arange(E)] = 1.0

    recvT = np.zeros((128, ETILES * NVAR), bf16)
    for et in range(ETILES):
        rr = edge_recv[et * 128:(et + 1) * 128]
        recvT[np.arange(128), et * NVAR + rr] = 1.0

    w = conv_weight.copy()
    w[np.arange(NVAR), np.arange(NVAR), K - 1] = 0.0
    wt = np.ascontiguousarray(w.transpose(1, 2, 0)).reshape(NVAR, K * NVAR).astype(bf16)

    bias_ones = np.concatenate(
        [conv_bias, np.ones(CHUNK, np.float32)]
    ).reshape(1, NVAR + CHUNK).astype(np.float32)

    dwb = dyn_weights.astype(bf16)  # [E, B, T-1, K]

    in_maps = []
    for core in range(NC_COUNT):
        b, h = divmod(core, 2)
        tau0 = 0 if h == 0 else TAU - L  # 0 or 1023
        xpad2 = np.zeros((NVAR, W_XPAD), np.float32)
        lo = tau0 - (K - 2)  # first x column needed
        src_lo = max(lo, 0)
        xpad2[:, src_lo - lo:W_XPAD - 1] = x[b, :, src_lo:tau0 + L + 1]
        a = dwb[:, b, tau0:tau0 + L, :]                  # [E, L, K]
        a = a.reshape(E, NCHUNK, CHUNK, K)               # [E, h2, tau, k]
        a = a.transpose(1, 0, 3, 2)                      # [h2, E, k, tau]
        dw = np.ascontiguousarray(a).reshape(NCHUNK * E, CHUNK * K)
        in_maps.append({
            "xpad": xpad2.astype(bf16),
            "dw": dw,
            "ssend": ssend,
            "wt": wt,
            "recvT": recvT,
            "bias_ones": bias_ones,
        })
    return in_maps


def _assemble(results):
    out = np.empty((B, TAU, NVAR, 1), np.float32)
    for core in range(NC_COUNT):
        b, h = divmod(core, 2)
        yT = results[core]["yT"]  # [NVAR, L] bf16
        if h == 0:
            out[b, 0:L, :, 0] = yT.T
        else:
            out[b, L:TAU, :, 0] = yT[:, 1:L].T
    return out


def run_on_hw(in_maps, trace=False, **kwargs):
    from concourse.bass_utils import run_bass_kernel_spmd

    nc = _get_program()
    return run_bass_kernel_spmd(
        nc, in_maps, core_ids=list(range(NC_COUNT)), trace=trace, **kwargs
    )


def kernel(spikes, conv_weight, conv_bias, dyn_weights, edge_send, edge_recv):
    in_maps = _host_prep(
        spikes, conv_weight, conv_bias, dyn_weights, edge_send, edge_recv
    )
    res = run_on_hw(in_maps)
    return _assemble(res.results)
